# revision 39
# baseline (speedup 1.0000x reference)
"""LocalWindowAttention (3x3 windows, B=16, 96x96, C=256, 4 heads) on 8
Trainium2 NeuronCores via Bass/Tile. Pure data parallel: 2 images per core.

v4 over v2: the per-round phase ordering (all qkv-proj, then all attention,
then all out-proj) head-blocked the PE queue on the Scalar/Vector evac+exp
+normalize chain (~40us/rep of PE gaps). Work is now emitted as a software
pipeline over 504-token chunks flattened across rounds: chunk c+1's qkv
projections are queued before chunk c's attention+out-proj, so the PE always
has dependency-free matmuls in front of it while Scalar/Vector catch up.
QK logits PSUM also double-buffers via a free-offset parity inside its two
banks, decoupling tile t+1's QK from tile t's exp.
"""

import numpy as np
import ml_dtypes

import concourse.bass as bass
import concourse.bacc as bacc
import concourse.tile as tile
from concourse import mybir
from concourse.bass_utils import run_bass_kernel_spmd

F32 = mybir.dt.float32
BF16 = mybir.dt.bfloat16
FP8 = mybir.dt.float8e4

B = 16
NCORES = 8
IMG = B // NCORES          # images per core
C = 256
NH = 4
HD = 64
WS = 3
GRID = 96
NSTRIP = 32                # window-rows per image
NT = GRID * GRID           # tokens per image
SCALE = HD ** -0.5


def _tiles_for(nstrips):
    nw = nstrips * 32          # windows in round
    full, rem = divmod(nw, 14)
    t = [14] * full
    if rem:
        t.append(rem)
    return t


def _round_plan(img=IMG):
    nstrips = img * NSTRIP
    plan = []
    s0 = 0
    while s0 < nstrips:
        ns = min(7, nstrips - s0)
        plan.append((s0, ns))
        s0 += ns
    return plan


class _Round:
    """Per-round geometry + views into manually double-buffered SBUF
    mega-tiles (parity = round index % 2).  Manual parity indexing instead
    of pool bufs=2 lets the same physical buffers be addressed identically
    every For_i iteration, so the next iteration's first chunk can be
    prefilled at the end of the body.  `partial` marks the tail round whose
    pad/zero regions are clobbered by full rounds sharing its parity and
    must be re-zeroed every iteration via `memsets()`."""

    def __init__(self, nc, mega, r, s0, ns):
        self.nc = nc
        self.s0, self.ns = s0, ns
        self.nt = ns * 288
        self.t0 = s0 * 288
        self.tiles = _tiles_for(ns)
        self.T = len(self.tiles)
        self.offs = np.cumsum([0] + [9 * w for w in self.tiles]).tolist()
        self.chunks = []
        for c0 in range(0, self.T, 4):
            tl = list(range(c0, min(c0 + 4, self.T)))
            self.chunks.append((self.offs[tl[0]], tl))
        self.partial = self.nt < 2016
        p = r % 2
        self.xT = mega["xT"][:, p]
        self.qT = mega["qT"][:, p]
        self.kT = mega["kT"][:, p]
        self.v_sb = mega["vs"][:, p]
        self.expm = mega["expm"][:, p]
        self.ao = mega["ao"][:, p]
        self.out_sb = mega["outs"][:, p]

    def memsets(self):
        pad = min(self.nt + 128, 2048)
        self.nc.vector.memset(self.kT[:, :, self.nt:pad], 0.0)
        self.nc.vector.memset(self.xT[:, :, self.nt:pad], 0.0)
        if self.tiles[-1] != 14:
            self.nc.vector.memset(self.expm[:, self.T - 1, :, :], 0.0)


def _build(nc, img=IMG, reps=1):
    x = nc.declare_dram_parameter("x", [2, img * NT, 128], BF16, isOutput=False).ap()
    wqkvT = nc.declare_dram_parameter("wqkvT", [128, 2, 768], BF16, isOutput=False).ap()
    wprojT = nc.declare_dram_parameter("wprojT", [128, 2, 256], BF16, isOutput=False).ap()
    maskc = nc.declare_dram_parameter("maskc", [128, 128], BF16, isOutput=False).ap()
    onesc = nc.declare_dram_parameter("onesc", [128, 64], BF16, isOutput=False).ap()
    ebias = nc.declare_dram_parameter("ebias", [128, 1], F32, isOutput=False).ap()
    y = nc.declare_dram_parameter("y", [img * NT, C], BF16, isOutput=True).ap()

    plan = _round_plan(img)

    with tile.TileContext(nc) as tc:
        with (
            tc.tile_pool(name="const", bufs=1) as constp,
            tc.tile_pool(name="sb", bufs=1) as sb,
            tc.tile_pool(name="ps", bufs=1, space="PSUM") as ps,
        ):
            wq_sb = constp.tile([128, 2, 768], BF16)
            nc.sync.dma_start(out=wq_sb[:], in_=wqkvT[:])
            wp_sb = constp.tile([128, 2, 256], BF16)
            nc.sync.dma_start(out=wp_sb[:], in_=wprojT[:])
            mask_sb = constp.tile([128, 128], BF16)
            nc.sync.dma_start(out=mask_sb[:], in_=maskc[:])
            ones_sb = constp.tile([128, 64], BF16)
            nc.sync.dma_start(out=ones_sb[:], in_=onesc[:])
            eb_sb = constp.tile([128, 1], F32)
            nc.sync.dma_start(out=eb_sb[:], in_=ebias[:])

            consts = (wq_sb, wp_sb, mask_sb, ones_sb, eb_sb)

            # Round state lives in manually double-buffered mega-tiles
            # (single pool slot each; parity dim indexed by round % 2).
            # Full rounds' pad zeroes survive forever (no instruction
            # writes them), and all full rounds share the two parities of
            # rounds 0/1, so memsetting those two here covers every full
            # round.  The partial tail round's pad region is clobbered by
            # full rounds each iteration, so it re-memsets in-loop.
            mega_shapes = {
                "xT": [128, 2, 2, 2048],
                "qT": [128, 2, 2, 2048],
                "kT": [128, 2, 2, 2048],
                "vs": [126, 2, 16, 256],
                "expm": [126, 2, 16, 4, 126],
                "ao": [128, 2, 2, 2048],
                "outs": [126, 2, 16, 256],
            }
            mega = {k: sb.tile(s, BF16, tag=k, bufs=1, name=f"mg_{k}")
                    for k, s in mega_shapes.items()}
            rounds = [_Round(nc, mega, r, s0, ns)
                      for r, (s0, ns) in enumerate(plan)]
            rounds[0].memsets()
            rounds[1].memsets()

            def _load(R):
                for cc in range(2):
                    nc.sync.dma_start(out=R.xT[:, cc, 0:R.nt],
                                      in_=x[cc, R.t0:R.t0 + R.nt, :],
                                      transpose=True)

            flat = []
            for r, R in enumerate(rounds):
                for ci in range(len(R.chunks)):
                    flat.append((r, ci))

            def qkv(i):
                r, ci = flat[i]
                R = rounds[r]
                ch = R.chunks[ci]
                qps = _q_mms(nc, ps, R, ch, consts)
                _q_evacs(nc, R, ch, qps)
                kps = _k_mms(nc, ps, R, ch, consts)
                vps = _v_mms(nc, ps, R, ch, consts)
                _kv_evacs(nc, R, ch, kps, vps)

            def _prologue():
                _load(rounds[0])
                qkv(0)

            def _body(tctr, trailing):
                # Flatten (round, chunk) and software-pipeline one chunk
                # deep.  Per steady-state step (chunk c):
                #   PE : QK(c) | q/k/v MMs(c+1) | out-proj(c-1) | denom/AV(c)
                #   S  : exp(c) | v,k evacs(c+1) | out-evac half(c-1)
                #   V  : q evacs(c+1) | out half(c-1) | mask/recip/norm(c)
                # so each engine's FIFO always has ready work while the
                # exp->mask->normalize chain of chunk c drains.  When
                # `trailing`, the next iteration's first x-load and qkv
                # chunk are emitted at the end of the body so they overlap
                # the pipeline drain before the For_i barrier.
                pending_proj = None
                for i, (r, ci) in enumerate(flat):
                    R = rounds[r]
                    ch = R.chunks[ci]
                    if ci == 0:
                        if r + 1 < len(rounds):
                            if rounds[r + 1].partial:
                                rounds[r + 1].memsets()
                            _load(rounds[r + 1])
                    _attn_part1(nc, ps, R, ch, consts, tctr)
                    if i + 1 < len(flat):
                        qkv(i + 1)
                    elif trailing:
                        _load(rounds[0])
                        qkv(0)
                    if pending_proj is not None:
                        pending_proj()
                    _attn_part2(nc, sb, ps, R, ch, consts, y)
                    pending_proj = _make_proj(nc, ps, R, ch, consts, y,
                                              last=(ci == len(R.chunks) - 1))
                pending_proj()

            tctr = [0]
            _prologue()
            if reps == 1:
                _body(tctr, trailing=False)
            else:
                with tc.For_i(0, reps, 1):
                    _body(tctr, trailing=True)
    return nc


def _evac(eng, nc, out, in_):
    if eng is nc.scalar:
        eng.copy(out=out, in_=in_)
    else:
        eng.tensor_copy(out=out, in_=in_)


def _proj_mms(nc, ps, R, ch, consts, base):
    """q (base=0) or k (base=256) projection matmuls for one chunk."""
    wq_sb = consts[0]
    f0, tl = ch
    nc_ = R.offs[tl[-1] + 1] - f0
    qps = []
    for mc in range(2):
        qp = ps.tile([128, 512], F32, tag="qk", bufs=2)
        for kc in range(2):
            nc.tensor.matmul(
                out=qp[:, 0:nc_],
                lhsT=wq_sb[:, kc, base + 128 * mc: base + 128 * mc + 128],
                rhs=R.xT[:, kc, f0:f0 + nc_],
                start=(kc == 0),
                stop=(kc == 1),
            )
        qps.append(qp)
    return qps


def _q_mms(nc, ps, R, ch, consts):
    return _proj_mms(nc, ps, R, ch, consts, 0)


def _k_mms(nc, ps, R, ch, consts):
    return _proj_mms(nc, ps, R, ch, consts, 256)


def _q_evacs(nc, R, ch, qps):
    f0, tl = ch
    nc_ = R.offs[tl[-1] + 1] - f0
    for mc in range(2):
        nc.vector.tensor_copy(out=R.qT[:, mc, f0:f0 + nc_],
                              in_=qps[mc][:, 0:nc_])


def _v_mms(nc, ps, R, ch, consts):
    wq_sb = consts[0]
    f0, tl = ch
    vps = []
    for tp_ in (tl[0], tl[0] + 2):
        pair = [t for t in (tp_, tp_ + 1) if t < R.T]
        if not pair:
            continue
        vp = ps.tile([128, 2, 256], F32, tag="vp", bufs=1)
        for j, t_ in enumerate(pair):
            tf = R.offs[t_]
            for kc in range(2):
                nc.tensor.matmul(
                    out=vp[:, j, 0:256],
                    lhsT=R.xT[:, kc, tf:tf + 128],
                    rhs=wq_sb[:, kc, 512:768],
                    start=(kc == 0),
                    stop=(kc == 1),
                )
        vps.append((tp_, pair, vp))
    return vps


def _kv_evacs(nc, R, ch, kps, vps):
    f0, tl = ch
    nc_ = R.offs[tl[-1] + 1] - f0
    for tp_, pair, vp in vps:
        if len(pair) == 2 and R.tiles[pair[1]] == 14:
            nc.scalar.copy(out=R.v_sb[0:126, tp_:tp_ + 2, :], in_=vp[0:126, :, :])
        else:
            for j, t_ in enumerate(pair):
                kb = 9 * R.tiles[t_]
                nc.scalar.copy(out=R.v_sb[0:kb, t_, :], in_=vp[0:kb, j, :])
    for mc in range(2):
        nc.scalar.copy(out=R.kT[:, mc, f0:f0 + nc_], in_=kps[mc][:, 0:nc_])


def _attn_part1(nc, ps, R, ch, consts, tctr):
    """QK^T logits per tile + exp -> expm [k, tile, (hh,mc), q].

    Row-group hh writes its own PSUM bank; within each hh bank tile parity
    rotates free offsets 0/252 so QK(t+1) does not wait for exp(t)."""
    eb_sb = consts[4]
    f0, tl = ch
    nc_ = R.offs[tl[-1] + 1] - f0
    last_ch = tl[-1] == R.T - 1
    aL = ps.tile([128, 2, 512], F32, tag="att", bufs=1)
    for t_ in tl:
        kb = 9 * R.tiles[t_]
        tf = R.offs[t_]
        po = 252 * (tctr[0] % 2)
        tctr[0] += 1
        # 128-col weight loads keep FWL enabled; fall back to kb cols only
        # when 128 would read into the next (not yet evacuated) chunk.
        m = 128 if (tf + 128 <= f0 + nc_ or last_ch) else kb
        for mc in range(2):
            for hh in range(2):
                p0 = 64 * hh
                nc.tensor.matmul(
                    out=aL[0:m, hh, po + 126 * mc: po + 126 * mc + kb],
                    lhsT=R.kT[p0:p0 + 64, mc, tf:tf + m],
                    rhs=R.qT[p0:p0 + 64, mc, tf:tf + kb],
                    start=True,
                    stop=True,
                )
        ein = bass.AP(tensor=aL.tensor, offset=aL.offset + po,
                      ap=[[aL.ap[0][0], kb], [512, 2], [126, 2], [1, kb]])
        nc.scalar.activation(
            out=R.expm[0:kb, t_, 0:4, 0:kb], in_=ein,
            func=mybir.ActivationFunctionType.Exp, scale=SCALE)


def _attn_part2(nc, sb, ps, R, ch, consts, y):
    wq_sb, wp_sb, mask_sb, ones_sb, eb_sb = consts
    f0, tl = ch
    nc_ = R.offs[tl[-1] + 1] - f0

    if True:
        # ---- mask: expm *= blockdiag(9) over this chunk ----
        m = mask_sb[0:126, 0:126]
        lo, hi = tl[0], tl[-1] + 1
        mb = bass.AP(tensor=m.tensor, offset=m.offset,
                     ap=[m.ap[0], [0, hi - lo], [0, 4], m.ap[1]])
        nc.vector.tensor_mul(
            out=R.expm[:, lo:hi, :, :], in0=R.expm[:, lo:hi, :, :], in1=mb)

        # ---- denominators (ones-matmul) + AV + normalize, per head-pair ----
        for Th in range(2):
            dp = ps.tile([128, 512], F32, tag="dp", bufs=1)
            for hh in range(2):
                h = 2 * Th + hh
                hc = 2 * (h % 2) + h // 2
                full = [t for t in tl if R.tiles[t] == 14]
                if full:
                    e0 = R.expm[0:126, full[0], hc, 0:126]
                    rhs = bass.AP(tensor=e0.tensor, offset=e0.offset,
                                  ap=[e0.ap[0], [4 * 126, len(full)],
                                      [1, 126]])
                    nc.tensor.matmul(
                        out=dp[64 * hh:64 * hh + 64, 0:126 * len(full)],
                        lhsT=ones_sb[0:126, 0:64],
                        rhs=rhs,
                        start=True, stop=True,
                        tile_position=(0, 64 * hh),
                    )
                for t in tl:
                    if R.tiles[t] == 14:
                        continue
                    kb = 9 * R.tiles[t]
                    nc.tensor.matmul(
                        out=dp[64 * hh:64 * hh + 64,
                               R.offs[t] - f0: R.offs[t] - f0 + kb],
                        lhsT=ones_sb[0:126, 0:64],
                        rhs=R.expm[0:126, t, hc, 0:kb],
                        start=True, stop=True,
                        tile_position=(0, 64 * hh),
                    )
            ap_ = ps.tile([128, 512], F32, tag="sm", bufs=2)
            for t in tl:
                kb = 9 * R.tiles[t]
                for hh in range(2):
                    h = 2 * Th + hh
                    hc = 2 * (h % 2) + h // 2
                    nc.tensor.matmul(
                        out=ap_[64 * hh:64 * hh + 64,
                                R.offs[t] - f0: R.offs[t] - f0 + kb],
                        lhsT=R.v_sb[0:kb, t, 64 * h: 64 * h + 64],
                        rhs=R.expm[0:kb, t, hc, 0:kb],
                        start=True, stop=True,
                        tile_position=(0, 64 * hh),
                    )
            rbc = sb.tile([128, 512], F32, tag="rbc", bufs=2)
            nc.vector.reciprocal_approx_fast(
                out=rbc[:, 0:nc_], in_=dp[:, 0:nc_])
            nc.vector.tensor_mul(out=R.ao[:, Th, f0:f0 + nc_],
                                 in0=ap_[:, 0:nc_],
                                 in1=rbc[:, 0:nc_])


def _make_proj(nc, ps, R, ch, consts, y, last):
    """Out-proj for chunk ch, emitted one pipeline step later so the
    normalize has a full step of slack before the PE needs ao."""
    wq_sb, wp_sb, mask_sb, ones_sb, eb_sb = consts
    f0, tl = ch
    nc_ = R.offs[tl[-1] + 1] - f0

    def emit():
        # ---- out-proj per tile pair + evac to strip-major out_sb ----
        for pi, tp_ in enumerate((tl[0], tl[0] + 2)):
            pair = [t for t in (tp_, tp_ + 1) if t < R.T]
            if not pair:
                continue
            op = ps.tile([128, 2, 256], F32, tag="sm", bufs=2)
            for j, t_ in enumerate(pair):
                tf = R.offs[t_]
                kb = 9 * R.tiles[t_]
                m = 128 if tf + 128 <= f0 + nc_ else kb
                for Th in range(2):
                    nc.tensor.matmul(
                        out=op[0:m, j, 0:256],
                        lhsT=R.ao[:, Th, tf:tf + m],
                        rhs=wp_sb[:, Th, :],
                        start=(Th == 0),
                        stop=(Th == 1),
                    )
            eng = nc.vector if pi == 0 else nc.scalar
            if len(pair) == 2 and R.tiles[pair[1]] == 14:
                _evac(eng, nc, R.out_sb[0:126, tp_:tp_ + 2, :], op[0:126, :, :])
            else:
                for j, t_ in enumerate(pair):
                    kb = 9 * R.tiles[t_]
                    _evac(eng, nc, R.out_sb[0:kb, t_, :], op[0:kb, j, :])

        if last:
            # ---- output DMA(s): token index = 126*t + p ----
            fullT = sum(1 for w in R.tiles if w == 14)
            yout = bass.AP(tensor=y.tensor, offset=R.t0 * C,
                           ap=[[C, 126], [126 * C, fullT], [1, C]])
            nc.sync.dma_start(out=yout, in_=R.out_sb[0:126, 0:fullT, 0:256])
            if fullT != R.T:
                kb = 9 * R.tiles[-1]
                ytail = bass.AP(tensor=y.tensor,
                                offset=(R.t0 + 126 * fullT) * C,
                                ap=[[C, kb], [1, C]])
                nc.sync.dma_start(out=ytail, in_=R.out_sb[0:kb, R.T - 1, 0:256])

    return emit


def _make_consts():
    bf16 = ml_dtypes.bfloat16
    mask = np.zeros((128, 128), np.float32)
    for p in range(126):
        for q in range(126):
            if p // 9 == q // 9:
                mask[p, q] = 1.0
    return {
        "maskc": mask.astype(bf16),
        "onesc": np.ones((128, 64), np.float32).astype(bf16),
        "ebias": np.full((128, 1), -2.5, np.float32),
    }


_NC_CACHE = {}


def _get_nc():
    if "nc" not in _NC_CACHE:
        nc = bacc.Bacc("TRN2", target_bir_lowering=False, debug=False,
                       num_devices=NCORES)
        _build(nc)
        nc.compile()
        _NC_CACHE["nc"] = nc
    return _NC_CACHE["nc"]


def _perm_x(x):
    """[B, 9216, 256] f32 raster -> [B, 2, 9216, 128] bf16 window-contiguous
    (col-major within each 3-row strip), split by channel half."""
    bf16 = ml_dtypes.bfloat16
    x = np.asarray(x, np.float32).reshape(B, NSTRIP, 3, GRID, C)
    x = x.transpose(0, 1, 3, 2, 4).reshape(B, NT, C)       # col-major tokens
    x = x.reshape(NCORES, IMG * NT, 2, 128).transpose(0, 2, 1, 3)
    return np.ascontiguousarray(x).astype(bf16)            # [cores, 2, img*NT, 128]


def _unperm_y(y):
    """[img, 9216, 256] bf16 col-major tokens -> [img, 9216, 256] f32 raster."""
    y = np.asarray(y, np.float32).reshape(-1, NSTRIP, GRID, 3, C)
    y = y.transpose(0, 1, 3, 2, 4).reshape(-1, NT, C)
    return y


def _in_maps(x, Wqkv, Wproj):
    bf16 = ml_dtypes.bfloat16
    consts = _make_consts()
    consts["wqkvT"] = np.ascontiguousarray(
        np.asarray(Wqkv, np.float32).T.reshape(2, 128, 768).transpose(1, 0, 2)
    ).astype(bf16)
    consts["wprojT"] = np.ascontiguousarray(
        np.asarray(Wproj, np.float32).T.reshape(2, 128, 256).transpose(1, 0, 2)
    ).astype(bf16)
    xp = _perm_x(x)
    return [{"x": xp[c], **consts} for c in range(NCORES)]


def kernel(x, Wqkv, Wproj, H, W):
    assert int(H) == GRID and int(W) == GRID
    nc = _get_nc()
    res = run_bass_kernel_spmd(nc, _in_maps(x, Wqkv, Wproj), list(range(NCORES)))
    out = np.concatenate([_unperm_y(res.results[c]["y"]) for c in range(NCORES)],
                         axis=0)
    return np.ascontiguousarray(out.reshape(B, NT, C)).astype(np.float32)
